# revision 2
# baseline (speedup 1.0000x reference)
"""Cost-volume kernel for Trainium2 (8 NeuronCores, batch-parallel).

out[b, k, h, w] = (1/(C*81)) * sum_c x[b,c,h,w] * warped[b,c,h+di,w+dj]
for the 81 offsets (di,dj) in [-4,4]^2 (zero-padded), B=8 -> one batch
element per core.

Device-side algorithm (per core), "col-tiled" with fine-grained pipeline:
  - the image is tiled into 4x8 x-tiles (M=32). One PSUM block [128,192]
    holds 4 adjacent tiles (same tile-row, 4 consecutive tile-cols) via
    4 PE *column-tiled* matmuls (tile_position=(0,32j)) on independent
    128x32 sub-arrays. lhsT = x-tile [C=128, 32] (tile-contiguous x),
    rhs = the tile's 12x16 window of the zero-padded warped image as a
    2D-strided AP [C, 12, 16].
  - warped is staged as 7 NON-overlapping row-bands of the padded image
    (edges 0/12/32/56/80/104/124/136). Small head/tail bands mean the
    first matmul only waits for a 0.79 MB load (~2 us) instead of a
    2.2 MB one (~22 us startup in the previous version). Windows that
    straddle band edges become partial-window matmuls writing disjoint
    PSUM column ranges (zero halo re-read).
  - x is staged in tile-row chunks (1/2/4/4/... tile-rows) so the first
    chunk is only 0.26 MB; chunk 0 rides scalar (parallel with w band 0
    on sync), the rest ride sync behind the w bands.
  - PSUM pool tiles of [128, 1024] (2 banks, bufs=4) hold 4 blocks
    (2 per 512-f32 bank at 0/192); one strided ACT/DVE scaled copy
    (alternating) drains 4 blocks -> one half of a [128, 1536] bf16
    dump tile; one DMA store per tile-row (32 stores of 393KB,
    alternating scalar/gpsimd). Finer drains cut the end-of-kernel
    drain backlog (~14 us tail -> ~3 us).
  - final relayout [81, H, W] is a constant-stride view on the host.

HBM traffic per core: x 8.4 + warped 9.2 (padded) + dump 12.6 = 30.2 MB.
"""

import numpy as np

B = 8
C, H, W = 128, 128, 256
R = 4
K = 2 * R + 1  # 9
NOFF = K * K  # 81
TH, TW = 4, 8  # x-tile shape (M = 32)
NH, NW = TH + 2 * R, TW + 2 * R  # window 12 x 16
N = NH * NW  # 192
SCALE = 1.0 / (C * NOFF)

NT_H, NT_W = H // TH, W // TW  # 32 x 32 tile grid
TPB = 4  # tiles per PSUM block (4 col-tiles)
BPG = 8  # blocks per store group (= one tile-row)
PW = W + 2 * R  # 264 padded cols
# Non-overlapping warped row-bands (no halo re-read). Windows that
# straddle a band edge become partial-window matmuls writing disjoint
# PSUM column ranges. Small first/last bands shorten startup/tail.
W_EDGES = [0, 12, 32, 56, 80, 104, 124, 136]
# x staged in tile-row chunks; tiny first chunk for fast start.
X_EDGES = [0, 1, 3, 7, 11, 15, 19, 23, 27, 32]
NGROUPS = NT_H  # one store per tile-row

PRECISION = "bf16"

_CACHE = {}


def _build_module(n_cores):
    import concourse.bacc as bacc
    import concourse.mybir as mybir
    import concourse.tile as tile

    dt = mybir.dt.bfloat16
    f32 = mybir.dt.float32
    # f32 offsets of the 4 blocks inside a [128, 1024] (2-bank) psum
    # tile: 2 blocks per 512-f32 bank at bank-internal 0/192.
    BLK_OFF = [(s // 2) * 512 + (s % 2) * 192 for s in range(4)]

    nc = bacc.Bacc(
        "TRN2", target_bir_lowering=False, debug=False, num_devices=n_cores
    )
    # x: tile-contiguous [C, nt_h, nt_w, TH*TW]; warped: padded row-major
    # [C, 136, 264]. Both host-prepped so every DMA is fully contiguous.
    x_d = nc.dram_tensor(
        "x", [C, NT_H * NT_W * TH * TW], dt, kind="ExternalInput"
    ).ap()
    w_d = nc.dram_tensor(
        "warped", [C, (H + 2 * R) * PW], dt, kind="ExternalInput"
    ).ap()
    out_d = nc.dram_tensor(
        "dump", [128, NGROUPS * BPG * N], dt, kind="ExternalOutput"
    ).ap()

    row_elems = NT_W * TH * TW  # 1024 x elems per tile-row per partition

    def wband_of(r):  # band index containing padded row r
        return max(i for i in range(len(W_EDGES) - 1) if W_EDGES[i] <= r)

    def xchunk_of(ltr):
        return max(i for i in range(len(X_EDGES) - 1) if X_EDGES[i] <= ltr)

    with tile.TileContext(nc) as tc:
        with (
            tc.tile_pool(name="wband", bufs=5) as wb_pool,
            tc.tile_pool(name="xband", bufs=4) as xb_pool,
            tc.tile_pool(name="dump", bufs=6) as dump_pool,
            tc.tile_pool(name="psum", bufs=4, space="PSUM") as psum_pool,
        ):
            store_engines = [nc.scalar, nc.gpsimd]
            wtiles = [None] * (len(W_EDGES) - 1)
            w2s = [None] * (len(W_EDGES) - 1)

            def get_wband(b):
                if wtiles[b] is None:
                    lo, hi = W_EDGES[b], W_EDGES[b + 1]
                    t = wb_pool.tile([128, (hi - lo) * PW], dt)
                    nc.sync.dma_start(out=t, in_=w_d[:, lo * PW : hi * PW])
                    wtiles[b] = t
                    w2s[b] = t[:].rearrange("p (h w) -> p h w", w=PW)
                return w2s[b]

            xtiles = [None] * (len(X_EDGES) - 1)

            def get_xchunk(c):
                if xtiles[c] is None:
                    lo, hi = X_EDGES[c], X_EDGES[c + 1]
                    t = xb_pool.tile([128, (hi - lo) * row_elems], dt)
                    # chunk 0 on scalar (parallel with w band 0 on sync at
                    # startup); later loads ride sync so stores never queue
                    # behind a prefetched load on the store rings.
                    eng = nc.scalar if c == 0 else nc.sync
                    eng.dma_start(
                        out=t,
                        in_=x_d[:, lo * row_elems : hi * row_elems],
                    )
                    xtiles[c] = t
                return xtiles[c]

            def prefetch_for(ltr):
                if ltr >= NT_H:
                    return
                r0 = ltr * TH
                get_wband(wband_of(r0))
                get_wband(wband_of(r0 + NH - 1))
                get_xchunk(xchunk_of(ltr))

            didx = 0  # drain index (2 per tile-row)
            for ltr in range(NT_H):
                r0 = ltr * TH
                # current tile-row's data (lazy, memoized)
                prefetch_for(ltr)
                xc = xchunk_of(ltr)
                xsb = get_xchunk(xc)
                # window row segments split by band edges
                segs = []
                r = r0
                while r < r0 + NH:
                    b = wband_of(r)
                    hi = min(r0 + NH, W_EDGES[b + 1])
                    segs.append((b, r, hi))
                    r = hi
                # prefetch next 2 tile-rows (hide band/chunk loads)
                prefetch_for(ltr + 1)
                prefetch_for(ltr + 2)

                db = dump_pool.tile([128, BPG * N], dt)
                for half in range(2):
                    ps = psum_pool.tile([128, 1024], f32)
                    for s in range(4):
                        q = half * 4 + s
                        for j in range(TPB):
                            itw = q * TPB + j
                            xoff = (
                                (ltr - X_EDGES[xc]) * NT_W + itw
                            ) * (TH * TW)
                            lhsT = xsb[:, xoff : xoff + TH * TW]
                            pj = ps[32 * j : 32 * (j + 1), :]
                            for b, lo, hi in segs:
                                w2 = get_wband(b)
                                rhs = w2[
                                    :,
                                    lo - W_EDGES[b] : hi - W_EDGES[b],
                                    itw * TW : itw * TW + NW,
                                ]
                                c0 = BLK_OFF[s] + (lo - r0) * NW
                                c1 = BLK_OFF[s] + (hi - r0) * NW
                                nc.tensor.matmul(
                                    pj[:, c0:c1],
                                    lhsT, rhs, start=True, stop=True,
                                    tile_position=(0, 32 * j),
                                )
                    # drain 4 blocks (2 banks) -> half of the dump tile
                    src2 = ps[:].rearrange("p (b x) -> p b x", b=2)[
                        :, :, 0 : 2 * N
                    ]
                    dst2 = db[:, half * 4 * N : (half + 1) * 4 * N].rearrange(
                        "p (b x) -> p b x", b=2
                    )
                    if didx % 2 == 0:
                        nc.scalar.mul(dst2, src2, SCALE)
                    else:
                        nc.vector.tensor_scalar_mul(dst2, src2, SCALE)
                    didx += 1
                eng = store_engines[ltr % len(store_engines)]
                eng.dma_start(
                    out=out_d[:, ltr * BPG * N : (ltr + 1) * BPG * N],
                    in_=db,
                )

    nc.compile()
    return nc


def _host_prep(x_b, warped_b):
    """x: [c,h,w] -> tile-contiguous [c, nt_h*nt_w*32]; warped -> padded
    row-major [c, 136*264]."""
    c = x_b.shape[0]
    xt = np.ascontiguousarray(
        x_b.reshape(c, NT_H, TH, NT_W, TW).transpose(0, 1, 3, 2, 4)
    ).reshape(c, NT_H * NT_W * TH * TW)
    wp = np.zeros((c, H + 2 * R, PW), dtype=x_b.dtype)
    wp[:, R : R + H, R : R + W] = warped_b
    return xt, wp.reshape(c, (H + 2 * R) * PW)


def _extract(dump):
    """[128, NGROUPS*BPG*N] -> [81, H, W] constant-stride view.

    dump element [m, g, s*N + n]:
      m = 32*j + hx*TW + wx ; n = (hx+di)*NW + (wx+dj)
      g = ltr ; s = q
      h = ltr*4 + hx ;  w = (q*4+j)*8 + wx
    """
    dmp = np.ascontiguousarray(dump).reshape(128, NGROUPS, BPG * N)
    sm, sg, sn = dmp.strides
    sn_e = sn  # innermost element stride (bytes)
    view = np.lib.stride_tricks.as_strided(
        dmp,
        shape=(K, K, NT_H, TH, 8, TPB, TW),
        #      di dj ltr   hx  q  j   wx
        strides=(
            NW * sn_e,            # di
            sn_e,                 # dj
            sg,                   # ltr
            TW * sm + NW * sn_e,  # hx
            N * sn_e,             # q (= s)
            32 * sm,              # j
            sm + sn_e,            # wx
        ),
    )
    # [di,dj, ltr,hx, q,j,wx] -> [81, H, W]
    out = np.ascontiguousarray(view).reshape(NOFF, H, W)
    return out.astype(np.float32)


def kernel(x, warped):
    from concourse import bass_utils

    x = np.asarray(x, dtype=np.float32)
    warped = np.asarray(warped, dtype=np.float32)
    assert x.shape == (B, C, H, W) and warped.shape == (B, C, H, W)

    import ml_dtypes

    x = x.astype(ml_dtypes.bfloat16)
    warped = warped.astype(ml_dtypes.bfloat16)

    key = "v4"
    if key not in _CACHE:
        _CACHE[key] = _build_module(B)
    nc = _CACHE[key]

    in_maps = []
    for b in range(B):
        xt, wp = _host_prep(x[b], warped[b])
        in_maps.append({"x": xt, "warped": wp})
    res = bass_utils.run_bass_kernel_spmd(nc, in_maps, core_ids=list(range(B)))
    global LAST_RESULTS
    LAST_RESULTS = res
    out = np.empty((B, NOFF, H, W), dtype=np.float32)
    for b in range(B):
        out[b] = _extract(res.results[b]["dump"])
    return out


# revision 3
# speedup vs baseline: 1.1166x; 1.1166x over previous
"""Cost-volume kernel for Trainium2 (8 NeuronCores, batch-parallel).

out[b, k, h, w] = (1/(C*81)) * sum_c x[b,c,h,w] * warped[b,c,h+di,w+dj]
for the 81 offsets (di,dj) in [-4,4]^2 (zero-padded), B=8 -> one batch
element per core.

Device-side algorithm (per core), "col-tiled" with fine-grained pipeline:
  - the image is tiled into 4x8 x-tiles (M=32). One PSUM block [128,192]
    holds 4 adjacent tiles (same tile-row, 4 consecutive tile-cols) via
    4 PE *column-tiled* matmuls (tile_position=(0,32j)) on independent
    128x32 sub-arrays. lhsT = x-tile [C=128, 32] (tile-contiguous x),
    rhs = the tile's 12x16 window of the zero-padded warped image as a
    2D-strided AP [C, 12, 16].
  - warped is staged as 7 NON-overlapping row-bands of the padded image
    (edges 0/12/32/56/80/104/124/136). Small head/tail bands mean the
    first matmul only waits for a 0.79 MB load (~2 us) instead of a
    2.2 MB one (~22 us startup in the previous version). Windows that
    straddle band edges become partial-window matmuls writing disjoint
    PSUM column ranges (zero halo re-read).
  - x is staged in tile-row chunks (1/2/4/4/... tile-rows) so the first
    chunk is only 0.26 MB; chunk 0 rides scalar (parallel with w band 0
    on sync), the rest ride sync behind the w bands.
  - PSUM pool tiles of [128, 1024] (2 banks, bufs=4) hold 4 blocks
    (2 per 512-f32 bank at 0/192); one strided ACT/DVE scaled copy
    (alternating) drains 4 blocks -> one half of a [128, 1536] bf16
    dump tile; one DMA store per tile-row (32 stores of 393KB,
    alternating scalar/gpsimd). Finer drains cut the end-of-kernel
    drain backlog (~14 us tail -> ~3 us).
  - final relayout [81, H, W] is a constant-stride view on the host.

HBM traffic per core: x 8.4 + warped 9.2 (padded) + dump 12.6 = 30.2 MB.
"""

import numpy as np

B = 8
C, H, W = 128, 128, 256
R = 4
K = 2 * R + 1  # 9
NOFF = K * K  # 81
TH, TW = 4, 8  # x-tile shape (M = 32)
NH, NW = TH + 2 * R, TW + 2 * R  # window 12 x 16
N = NH * NW  # 192
SCALE = 1.0 / (C * NOFF)

NT_H, NT_W = H // TH, W // TW  # 32 x 32 tile grid
TPB = 4  # tiles per PSUM block (4 col-tiles)
BPG = 8  # blocks per store group (= one tile-row)
PW = W + 2 * R  # 264 padded cols
# Non-overlapping warped row-bands (no halo re-read). Windows that
# straddle a band edge become partial-window matmuls writing disjoint
# PSUM column ranges. Small first/last bands shorten startup/tail.
W_EDGES = [0, 12, 32, 56, 80, 104, 124, 136]
# x staged in tile-row chunks; tiny first chunk for fast start.
X_EDGES = [0, 1, 3, 7, 11, 15, 19, 23, 27, 31, 32]
NGROUPS = NT_H  # one store per tile-row

PRECISION = "bf16"

_CACHE = {}


def _build_module(n_cores):
    import concourse.bacc as bacc
    import concourse.mybir as mybir
    import concourse.tile as tile

    dt = mybir.dt.bfloat16
    f32 = mybir.dt.float32
    # f32 offsets of the 4 blocks inside a [128, 1024] (2-bank) psum
    # tile: 2 blocks per 512-f32 bank at bank-internal 0/192.
    BLK_OFF = [(s // 2) * 512 + (s % 2) * 192 for s in range(4)]

    nc = bacc.Bacc(
        "TRN2", target_bir_lowering=False, debug=False, num_devices=n_cores
    )
    # x: tile-contiguous [C, nt_h, nt_w, TH*TW]; warped: padded row-major
    # [C, 136, 264]. Both host-prepped so every DMA is fully contiguous.
    x_d = nc.dram_tensor(
        "x", [C, NT_H * NT_W * TH * TW], dt, kind="ExternalInput"
    ).ap()
    w_d = nc.dram_tensor(
        "warped", [C, (H + 2 * R) * PW], dt, kind="ExternalInput"
    ).ap()
    out_d = nc.dram_tensor(
        "dump", [128, NGROUPS * BPG * N], dt, kind="ExternalOutput"
    ).ap()

    row_elems = NT_W * TH * TW  # 1024 x elems per tile-row per partition

    def wband_of(r):  # band index containing padded row r
        return max(i for i in range(len(W_EDGES) - 1) if W_EDGES[i] <= r)

    def xchunk_of(ltr):
        return max(i for i in range(len(X_EDGES) - 1) if X_EDGES[i] <= ltr)

    with tile.TileContext(nc) as tc:
        with (
            tc.tile_pool(name="wband", bufs=7) as wb_pool,
            tc.tile_pool(name="xband", bufs=6) as xb_pool,
            tc.tile_pool(name="dump", bufs=6) as dump_pool,
            tc.tile_pool(name="psum", bufs=4, space="PSUM") as psum_pool,
        ):
            store_engines = [nc.scalar, nc.gpsimd]
            wtiles = [None] * (len(W_EDGES) - 1)
            w2s = [None] * (len(W_EDGES) - 1)

            def get_wband(b):
                if wtiles[b] is None:
                    lo, hi = W_EDGES[b], W_EDGES[b + 1]
                    t = wb_pool.tile([128, (hi - lo) * PW], dt)
                    nc.sync.dma_start(out=t, in_=w_d[:, lo * PW : hi * PW])
                    wtiles[b] = t
                    w2s[b] = t[:].rearrange("p (h w) -> p h w", w=PW)
                return w2s[b]

            xtiles = [None] * (len(X_EDGES) - 1)

            def get_xchunk(c):
                if xtiles[c] is None:
                    lo, hi = X_EDGES[c], X_EDGES[c + 1]
                    t = xb_pool.tile([128, (hi - lo) * row_elems], dt)
                    # chunk 0 on scalar (parallel with w band 0 on sync at
                    # startup); later loads ride sync so stores never queue
                    # behind a prefetched load on the store rings.
                    eng = nc.scalar if c == 0 else nc.sync
                    eng.dma_start(
                        out=t,
                        in_=x_d[:, lo * row_elems : hi * row_elems],
                    )
                    xtiles[c] = t
                return xtiles[c]

            def prefetch_for(ltr):
                if ltr >= NT_H:
                    return
                r0 = ltr * TH
                get_wband(wband_of(r0))
                get_wband(wband_of(r0 + NH - 1))
                get_xchunk(xchunk_of(ltr))

            didx = 0  # drain index (2 per tile-row)
            for ltr in range(NT_H):
                r0 = ltr * TH
                # current tile-row's data (lazy, memoized)
                prefetch_for(ltr)
                xc = xchunk_of(ltr)
                xsb = get_xchunk(xc)
                # window row segments split by band edges
                segs = []
                r = r0
                while r < r0 + NH:
                    b = wband_of(r)
                    hi = min(r0 + NH, W_EDGES[b + 1])
                    segs.append((b, r, hi))
                    r = hi
                # deep prefetch (hide band/chunk loads behind ~6 rows
                # of compute; pool bufs throttle actual SBUF residency)
                for d in range(1, 7):
                    prefetch_for(ltr + d)

                db = dump_pool.tile([128, BPG * N], dt)
                for half in range(2):
                    ps = psum_pool.tile([128, 1024], f32)
                    for s in range(4):
                        q = half * 4 + s
                        for j in range(TPB):
                            itw = q * TPB + j
                            xoff = (
                                (ltr - X_EDGES[xc]) * NT_W + itw
                            ) * (TH * TW)
                            lhsT = xsb[:, xoff : xoff + TH * TW]
                            pj = ps[32 * j : 32 * (j + 1), :]
                            for b, lo, hi in segs:
                                w2 = get_wband(b)
                                rhs = w2[
                                    :,
                                    lo - W_EDGES[b] : hi - W_EDGES[b],
                                    itw * TW : itw * TW + NW,
                                ]
                                c0 = BLK_OFF[s] + (lo - r0) * NW
                                c1 = BLK_OFF[s] + (hi - r0) * NW
                                nc.tensor.matmul(
                                    pj[:, c0:c1],
                                    lhsT, rhs, start=True, stop=True,
                                    tile_position=(0, 32 * j),
                                )
                    # drain 4 blocks (2 banks) -> half of the dump tile
                    src2 = ps[:].rearrange("p (b x) -> p b x", b=2)[
                        :, :, 0 : 2 * N
                    ]
                    dst2 = db[:, half * 4 * N : (half + 1) * 4 * N].rearrange(
                        "p (b x) -> p b x", b=2
                    )
                    # half0 -> vector, half1 -> scalar: the store that
                    # follows on scalar then only waits on its own
                    # engine's preceding drain (in-order) plus an
                    # already-finished vector drain.
                    if half == 0:
                        nc.vector.tensor_scalar_mul(dst2, src2, SCALE)
                    else:
                        nc.scalar.mul(dst2, src2, SCALE)
                    didx += 1
                eng = store_engines[ltr % len(store_engines)]
                eng.dma_start(
                    out=out_d[:, ltr * BPG * N : (ltr + 1) * BPG * N],
                    in_=db,
                )

    nc.compile()
    return nc


def _host_prep(x_b, warped_b):
    """x: [c,h,w] -> tile-contiguous [c, nt_h*nt_w*32]; warped -> padded
    row-major [c, 136*264]."""
    c = x_b.shape[0]
    xt = np.ascontiguousarray(
        x_b.reshape(c, NT_H, TH, NT_W, TW).transpose(0, 1, 3, 2, 4)
    ).reshape(c, NT_H * NT_W * TH * TW)
    wp = np.zeros((c, H + 2 * R, PW), dtype=x_b.dtype)
    wp[:, R : R + H, R : R + W] = warped_b
    return xt, wp.reshape(c, (H + 2 * R) * PW)


def _extract(dump):
    """[128, NGROUPS*BPG*N] -> [81, H, W] constant-stride view.

    dump element [m, g, s*N + n]:
      m = 32*j + hx*TW + wx ; n = (hx+di)*NW + (wx+dj)
      g = ltr ; s = q
      h = ltr*4 + hx ;  w = (q*4+j)*8 + wx
    """
    dmp = np.ascontiguousarray(dump).reshape(128, NGROUPS, BPG * N)
    sm, sg, sn = dmp.strides
    sn_e = sn  # innermost element stride (bytes)
    view = np.lib.stride_tricks.as_strided(
        dmp,
        shape=(K, K, NT_H, TH, 8, TPB, TW),
        #      di dj ltr   hx  q  j   wx
        strides=(
            NW * sn_e,            # di
            sn_e,                 # dj
            sg,                   # ltr
            TW * sm + NW * sn_e,  # hx
            N * sn_e,             # q (= s)
            32 * sm,              # j
            sm + sn_e,            # wx
        ),
    )
    # [di,dj, ltr,hx, q,j,wx] -> [81, H, W]
    out = np.ascontiguousarray(view).reshape(NOFF, H, W)
    return out.astype(np.float32)


def kernel(x, warped):
    from concourse import bass_utils

    x = np.asarray(x, dtype=np.float32)
    warped = np.asarray(warped, dtype=np.float32)
    assert x.shape == (B, C, H, W) and warped.shape == (B, C, H, W)

    import ml_dtypes

    x = x.astype(ml_dtypes.bfloat16)
    warped = warped.astype(ml_dtypes.bfloat16)

    key = "v5"
    if key not in _CACHE:
        _CACHE[key] = _build_module(B)
    nc = _CACHE[key]

    in_maps = []
    for b in range(B):
        xt, wp = _host_prep(x[b], warped[b])
        in_maps.append({"x": xt, "warped": wp})
    res = bass_utils.run_bass_kernel_spmd(nc, in_maps, core_ids=list(range(B)))
    global LAST_RESULTS
    LAST_RESULTS = res
    out = np.empty((B, NOFF, H, W), dtype=np.float32)
    for b in range(B):
        out[b] = _extract(res.results[b]["dump"])
    return out


# revision 7
# speedup vs baseline: 1.1484x; 1.0284x over previous
"""Cost-volume kernel for Trainium2 (8 NeuronCores, batch-parallel).

out[b, k, h, w] = (1/(C*81)) * sum_c x[b,c,h,w] * warped[b,c,h+di,w+dj]
for the 81 offsets (di,dj) in [-4,4]^2 (zero-padded), B=8 -> one batch
element per core.

Device-side algorithm (per core):
  - the image is tiled into 4x8 x-tiles (M=32). One PSUM block [128,192]
    holds 4 adjacent tiles (same tile-row, 4 consecutive tile-cols) via
    4 PE *column-tiled* matmuls (tile_position=(0,32j)) on independent
    128x32 sub-arrays; the 4 streams pipeline so a block costs ~170ns
    of PE issue regardless of splits. lhsT = x-tile [C=128, 32]
    (tile-contiguous x), rhs = the tile's 12x16 window of the
    zero-padded warped image as a 2D-strided AP [C, 12, 16].
  - warped is staged as 5 NON-overlapping row-bands of the padded image
    (edges 0/12/44/76/108/136; zero halo re-read). Windows straddling a
    band edge become two partial-window matmuls writing disjoint PSUM
    column ranges (~free on the PE quad pipeline). The tiny first band
    (0.79 MB) gets the first matmul going ~2 us after loads start.
  - x is staged in tile-row chunks (1/2/4/... tile-rows); chunk 0 rides
    scalar (parallel with w band 0 on sync); the rest ride sync.
    All loads are issued ~6 tile-rows ahead (pool bufs throttle).
  - PSUM pool tiles of [128, 512] (1 bank, bufs=8) hold 2 blocks at
    f32 offsets 0/192; one contiguous scaled copy per bank drains
    [128, 384] -> a quarter of a [128, 1536] bf16 dump tile (bufs=12).
    All 4 drains of a tile-row ride ONE engine (DVE for even rows, ACT
    for odd) so the store waits on a single engine; stores pair with
    the drain engine (gpsimd for DVE rows, scalar for ACT rows, which
    makes the store same-engine in-order with its drains).
  - final relayout [81, H, W] is a constant-stride view on the host.

HBM traffic per core: x 8.4 + warped 9.2 (padded) + dump 12.6 = 30.2 MB
at the ~425 GB/s fabric rate, plus ~7 us fixed preamble.
"""

import numpy as np

B = 8
C, H, W = 128, 128, 256
R = 4
K = 2 * R + 1  # 9
NOFF = K * K  # 81
TH, TW = 4, 8  # x-tile shape (M = 32)
NH, NW = TH + 2 * R, TW + 2 * R  # window 12 x 16
N = NH * NW  # 192
SCALE = 1.0 / (C * NOFF)

NT_H, NT_W = H // TH, W // TW  # 32 x 32 tile grid
TPB = 4  # tiles per PSUM block (4 col-tiles)
BPG = 8  # blocks per store group (= one tile-row)
PW = W + 2 * R  # 264 padded cols
# Non-overlapping warped row-bands (zero halo re-read); windows that
# straddle an edge become two partial-window matmuls. Tiny first band
# for fast startup.
W_EDGES = [0, 12, 44, 76, 108, 136]
# x staged in tile-row chunks; tiny first chunk for fast start.
X_EDGES = [0, 1, 3, 7, 11, 15, 19, 23, 27, 31, 32]
NGROUPS = NT_H  # one store per tile-row

PRECISION = "bf16"

_CACHE = {}


def _build_module(n_cores):
    import concourse.bacc as bacc
    import concourse.mybir as mybir
    import concourse.tile as tile

    dt = mybir.dt.bfloat16
    f32 = mybir.dt.float32
    # f32 offsets of the 2 blocks inside a [128, 512] (1-bank) psum
    # tile, at bank-internal 0/192.
    BLK_OFF = [0, 192]

    nc = bacc.Bacc(
        "TRN2", target_bir_lowering=False, debug=False, num_devices=n_cores
    )
    # x: tile-contiguous [C, nt_h, nt_w, TH*TW]; warped: padded row-major
    # [C, 136, 264]. Both host-prepped so every DMA is fully contiguous.
    x_d = nc.dram_tensor(
        "x", [C, NT_H * NT_W * TH * TW], dt, kind="ExternalInput"
    ).ap()
    w_d = nc.dram_tensor(
        "warped", [C, (H + 2 * R) * PW], dt, kind="ExternalInput"
    ).ap()
    out_d = nc.dram_tensor(
        "dump", [128, NGROUPS * BPG * N], dt, kind="ExternalOutput"
    ).ap()

    row_elems = NT_W * TH * TW  # 1024 x elems per tile-row per partition

    def wband_of(r):  # band index containing padded row r
        return max(i for i in range(len(W_EDGES) - 1) if W_EDGES[i] <= r)

    def xchunk_of(ltr):
        return max(i for i in range(len(X_EDGES) - 1) if X_EDGES[i] <= ltr)

    with tile.TileContext(nc) as tc:
        with (
            tc.tile_pool(name="wband", bufs=4) as wb_pool,
            tc.tile_pool(name="xband", bufs=6) as xb_pool,
            tc.tile_pool(name="dump", bufs=12) as dump_pool,
            tc.tile_pool(name="psum", bufs=8, space="PSUM") as psum_pool,
        ):
            wtiles = [None] * (len(W_EDGES) - 1)
            w2s = [None] * (len(W_EDGES) - 1)

            def get_wband(b):
                if wtiles[b] is None:
                    lo, hi = W_EDGES[b], W_EDGES[b + 1]
                    t = wb_pool.tile([128, (hi - lo) * PW], dt)
                    nc.sync.dma_start(out=t, in_=w_d[:, lo * PW : hi * PW])
                    wtiles[b] = t
                    w2s[b] = t[:].rearrange("p (h w) -> p h w", w=PW)
                return w2s[b]

            xtiles = [None] * (len(X_EDGES) - 1)

            def get_xchunk(c):
                if xtiles[c] is None:
                    lo, hi = X_EDGES[c], X_EDGES[c + 1]
                    t = xb_pool.tile([128, (hi - lo) * row_elems], dt)
                    # chunk 0 on scalar (parallel with w band 0 on sync
                    # at startup); later loads ride sync.
                    eng = nc.scalar if c == 0 else nc.sync
                    eng.dma_start(
                        out=t,
                        in_=x_d[:, lo * row_elems : hi * row_elems],
                    )
                    xtiles[c] = t
                return xtiles[c]

            def prefetch_for(ltr):
                if ltr >= NT_H:
                    return
                r0 = ltr * TH
                get_wband(wband_of(r0))
                get_wband(wband_of(r0 + NH - 1))
                get_xchunk(xchunk_of(ltr))

            for ltr in range(NT_H):
                r0 = ltr * TH
                prefetch_for(ltr)
                xc = xchunk_of(ltr)
                xsb = get_xchunk(xc)
                # window row segments split by band edges
                segs = []
                r = r0
                while r < r0 + NH:
                    b = wband_of(r)
                    hi = min(r0 + NH, W_EDGES[b + 1])
                    segs.append((b, r, hi))
                    r = hi
                # deep prefetch (hide band/chunk loads behind ~6 rows of
                # compute; pool bufs throttle actual SBUF residency)
                for d in range(1, 7):
                    prefetch_for(ltr + d)

                db = dump_pool.tile([128, BPG * N], dt)
                for bank in range(4):
                    ps = psum_pool.tile([128, 512], f32)
                    for s in range(2):
                        q = bank * 2 + s
                        for j in range(TPB):
                            itw = q * TPB + j
                            xoff = (
                                (ltr - X_EDGES[xc]) * NT_W + itw
                            ) * (TH * TW)
                            lhsT = xsb[:, xoff : xoff + TH * TW]
                            pj = ps[32 * j : 32 * (j + 1), :]
                            for b, lo, hi in segs:
                                w2 = get_wband(b)
                                rhs = w2[
                                    :,
                                    lo - W_EDGES[b] : hi - W_EDGES[b],
                                    itw * TW : itw * TW + NW,
                                ]
                                c0 = BLK_OFF[s] + (lo - r0) * NW
                                c1 = BLK_OFF[s] + (hi - r0) * NW
                                nc.tensor.matmul(
                                    pj[:, c0:c1],
                                    lhsT, rhs, start=True, stop=True,
                                    tile_position=(0, 32 * j),
                                )
                    # drain 2 blocks (1 bank, contiguous 384 f32) into a
                    # quarter of the dump tile; one engine per tile-row.
                    src1 = ps[:, 0 : 2 * N]
                    dst1 = db[:, bank * 2 * N : (bank + 1) * 2 * N]
                    if ltr % 2 == 0:
                        nc.vector.tensor_scalar_mul(dst1, src1, SCALE)
                    else:
                        nc.scalar.mul(dst1, src1, SCALE)
                # store pairs with the drain engine: ACT rows store on
                # scalar (same engine, in-order after its drains); DVE
                # rows store on gpsimd.
                eng = nc.gpsimd if ltr % 2 == 0 else nc.scalar
                eng.dma_start(
                    out=out_d[:, ltr * BPG * N : (ltr + 1) * BPG * N],
                    in_=db,
                )

    nc.compile()
    return nc


def _host_prep(x_b, warped_b):
    """x: [c,h,w] -> tile-contiguous [c, nt_h*nt_w*32]; warped -> padded
    row-major [c, 136*264]."""
    c = x_b.shape[0]
    xt = np.ascontiguousarray(
        x_b.reshape(c, NT_H, TH, NT_W, TW).transpose(0, 1, 3, 2, 4)
    ).reshape(c, NT_H * NT_W * TH * TW)
    wp = np.zeros((c, H + 2 * R, PW), dtype=x_b.dtype)
    wp[:, R : R + H, R : R + W] = warped_b
    return xt, wp.reshape(c, (H + 2 * R) * PW)


def _extract(dump):
    """[128, NGROUPS*BPG*N] -> [81, H, W] constant-stride view.

    dump element [m, g, s*N + n]:
      m = 32*j + hx*TW + wx ; n = (hx+di)*NW + (wx+dj)
      g = ltr ; s = q
      h = ltr*4 + hx ;  w = (q*4+j)*8 + wx
    """
    dmp = np.ascontiguousarray(dump).reshape(128, NGROUPS, BPG * N)
    sm, sg, sn = dmp.strides
    sn_e = sn  # innermost element stride (bytes)
    view = np.lib.stride_tricks.as_strided(
        dmp,
        shape=(K, K, NT_H, TH, 8, TPB, TW),
        #      di dj ltr   hx  q  j   wx
        strides=(
            NW * sn_e,            # di
            sn_e,                 # dj
            sg,                   # ltr
            TW * sm + NW * sn_e,  # hx
            N * sn_e,             # q (= s)
            32 * sm,              # j
            sm + sn_e,            # wx
        ),
    )
    # [di,dj, ltr,hx, q,j,wx] -> [81, H, W]
    out = np.ascontiguousarray(view).reshape(NOFF, H, W)
    return out.astype(np.float32)


def kernel(x, warped):
    from concourse import bass_utils

    x = np.asarray(x, dtype=np.float32)
    warped = np.asarray(warped, dtype=np.float32)
    assert x.shape == (B, C, H, W) and warped.shape == (B, C, H, W)

    import ml_dtypes

    x = x.astype(ml_dtypes.bfloat16)
    warped = warped.astype(ml_dtypes.bfloat16)

    key = "v9"
    if key not in _CACHE:
        _CACHE[key] = _build_module(B)
    nc = _CACHE[key]

    in_maps = []
    for b in range(B):
        xt, wp = _host_prep(x[b], warped[b])
        in_maps.append({"x": xt, "warped": wp})
    res = bass_utils.run_bass_kernel_spmd(nc, in_maps, core_ids=list(range(B)))
    global LAST_RESULTS
    LAST_RESULTS = res
    out = np.empty((B, NOFF, H, W), dtype=np.float32)
    for b in range(B):
        out[b] = _extract(res.results[b]["dump"])
    return out


# revision 12
# speedup vs baseline: 1.1587x; 1.0090x over previous
"""Cost-volume kernel for Trainium2 (8 NeuronCores, batch-parallel).

out[b, k, h, w] = (1/(C*81)) * sum_c x[b,c,h,w] * warped[b,c,h+di,w+dj]
for the 81 offsets (di,dj) in [-4,4]^2 (zero-padded), B=8 -> one batch
element per core.

Device-side algorithm (per core):
  - the image is tiled into 4x8 x-tiles (M=32). One PSUM block [128,192]
    holds 4 adjacent tiles (same tile-row, 4 consecutive tile-cols) via
    4 PE *column-tiled* matmuls (tile_position=(0,32j)) on independent
    128x32 sub-arrays; the 4 streams pipeline so a block costs ~170ns
    of PE issue regardless of splits. lhsT = x-tile [C=128, 32]
    (tile-contiguous x), rhs = the tile's 12x16 window of the
    zero-padded warped image as a 2D-strided AP [C, 12, 16].
  - warped is staged as 5 NON-overlapping row-bands of the padded image
    (edges 0/12/44/76/108/136; zero halo re-read). Windows straddling a
    band edge become two partial-window matmuls writing disjoint PSUM
    column ranges (~free on the PE quad pipeline). The tiny first band
    (0.79 MB) gets the first matmul going ~2 us after loads start.
  - x is staged in tile-row chunks (1/2/4/... tile-rows); chunk 0 rides
    scalar (parallel with w band 0 on sync); the rest ride sync.
    All loads are issued ~6 tile-rows ahead (pool bufs throttle).
  - PSUM pool tiles of [128, 512] (1 bank, bufs=8) hold 2 blocks at
    f32 offsets 0/192; one contiguous scaled copy per bank drains
    [128, 384] -> a quarter of a [128, 1536] bf16 dump tile (bufs=12).
    All 4 drains of a tile-row ride ONE engine (DVE for even rows, ACT
    for odd) so the store waits on a single engine; stores pair with
    the drain engine (gpsimd for DVE rows, scalar for ACT rows, which
    makes the store same-engine in-order with its drains).
  - final relayout [81, H, W] is a constant-stride view on the host.

HBM traffic per core: x 8.4 + warped 9.2 (padded) + dump 12.6 = 30.2 MB
at the ~425 GB/s fabric rate, plus ~7 us fixed preamble.
"""

import numpy as np

B = 8
C, H, W = 128, 128, 256
R = 4
K = 2 * R + 1  # 9
NOFF = K * K  # 81
TH, TW = 4, 8  # x-tile shape (M = 32)
NH, NW = TH + 2 * R, TW + 2 * R  # window 12 x 16
N = NH * NW  # 192
SCALE = 1.0 / (C * NOFF)

NT_H, NT_W = H // TH, W // TW  # 32 x 32 tile grid
TPB = 4  # tiles per PSUM block (4 col-tiles)
BPG = 8  # blocks per store group (= one tile-row)
PW = W + 2 * R  # 264 padded cols
# Non-overlapping warped row-bands (zero halo re-read); windows that
# straddle an edge become two partial-window matmuls. Tiny first band
# for fast startup.
W_EDGES = [0, 12, 44, 76, 108, 136]
# x staged in tile-row chunks; tiny first chunk for fast start.
X_EDGES = [0, 1, 3, 7, 11, 15, 19, 23, 27, 31, 32]
NGROUPS = NT_H  # one store per tile-row

PRECISION = "bf16"

_CACHE = {}


def _build_module(n_cores):
    import concourse.bacc as bacc
    import concourse.mybir as mybir
    import concourse.tile as tile

    dt = mybir.dt.bfloat16
    f32 = mybir.dt.float32
    # f32 offsets of the 2 blocks inside a [128, 512] (1-bank) psum
    # tile, at bank-internal 0/192.
    BLK_OFF = [0, 192]

    nc = bacc.Bacc(
        "TRN2", target_bir_lowering=False, debug=False, num_devices=n_cores
    )
    # x: tile-contiguous [C, nt_h, nt_w, TH*TW]; warped: padded row-major
    # [C, 136, 264]. Both host-prepped so every DMA is fully contiguous.
    x_d = nc.dram_tensor(
        "x", [C, NT_H * NT_W * TH * TW], dt, kind="ExternalInput"
    ).ap()
    w_d = nc.dram_tensor(
        "warped", [C, (H + 2 * R) * PW], dt, kind="ExternalInput"
    ).ap()
    out_d = nc.dram_tensor(
        "dump", [128, NGROUPS * BPG * N], dt, kind="ExternalOutput"
    ).ap()

    row_elems = NT_W * TH * TW  # 1024 x elems per tile-row per partition

    def wband_of(r):  # band index containing padded row r
        return max(i for i in range(len(W_EDGES) - 1) if W_EDGES[i] <= r)

    def xchunk_of(ltr):
        return max(i for i in range(len(X_EDGES) - 1) if X_EDGES[i] <= ltr)

    with tile.TileContext(nc) as tc:
        with (
            tc.tile_pool(name="wband", bufs=4) as wb_pool,
            tc.tile_pool(name="xband", bufs=6) as xb_pool,
            tc.tile_pool(name="dump", bufs=12) as dump_pool,
            tc.tile_pool(name="psum", bufs=8, space="PSUM") as psum_pool,
        ):
            wtiles = [None] * (len(W_EDGES) - 1)
            w2s = [None] * (len(W_EDGES) - 1)

            def get_wband(b):
                if wtiles[b] is None:
                    lo, hi = W_EDGES[b], W_EDGES[b + 1]
                    t = wb_pool.tile([128, (hi - lo) * PW], dt)
                    nc.sync.dma_start(out=t, in_=w_d[:, lo * PW : hi * PW])
                    wtiles[b] = t
                    w2s[b] = t[:].rearrange("p (h w) -> p h w", w=PW)
                return w2s[b]

            xtiles = [None] * (len(X_EDGES) - 1)

            def get_xchunk(c):
                if xtiles[c] is None:
                    lo, hi = X_EDGES[c], X_EDGES[c + 1]
                    t = xb_pool.tile([128, (hi - lo) * row_elems], dt)
                    # chunk 0 on scalar (parallel with w band 0 on sync
                    # at startup); later loads ride sync.
                    eng = nc.scalar if c == 0 else nc.sync
                    eng.dma_start(
                        out=t,
                        in_=x_d[:, lo * row_elems : hi * row_elems],
                    )
                    xtiles[c] = t
                return xtiles[c]

            def prefetch_for(ltr):
                if ltr >= NT_H:
                    return
                r0 = ltr * TH
                get_wband(wband_of(r0))
                get_wband(wband_of(r0 + NH - 1))
                get_xchunk(xchunk_of(ltr))

            for ltr in range(NT_H):
                r0 = ltr * TH
                prefetch_for(ltr)
                xc = xchunk_of(ltr)
                xsb = get_xchunk(xc)
                # window row segments split by band edges
                segs = []
                r = r0
                while r < r0 + NH:
                    b = wband_of(r)
                    hi = min(r0 + NH, W_EDGES[b + 1])
                    segs.append((b, r, hi))
                    r = hi
                # deep prefetch (hide band/chunk loads behind ~6 rows of
                # compute; pool bufs throttle actual SBUF residency)
                for d in range(1, 7):
                    prefetch_for(ltr + d)

                db = dump_pool.tile([128, BPG * N], dt)
                for bank in range(4):
                    ps = psum_pool.tile([128, 512], f32)
                    for s in range(2):
                        q = bank * 2 + s
                        for j in range(TPB):
                            itw = q * TPB + j
                            xoff = (
                                (ltr - X_EDGES[xc]) * NT_W + itw
                            ) * (TH * TW)
                            lhsT = xsb[:, xoff : xoff + TH * TW]
                            pj = ps[32 * j : 32 * (j + 1), :]
                            for b, lo, hi in segs:
                                w2 = get_wband(b)
                                rhs = w2[
                                    :,
                                    lo - W_EDGES[b] : hi - W_EDGES[b],
                                    itw * TW : itw * TW + NW,
                                ]
                                c0 = BLK_OFF[s] + (lo - r0) * NW
                                c1 = BLK_OFF[s] + (hi - r0) * NW
                                nc.tensor.matmul(
                                    pj[:, c0:c1],
                                    lhsT, rhs, start=True, stop=True,
                                    tile_position=(0, 32 * j),
                                )
                    # drain 2 blocks (1 bank, contiguous 384 f32) into a
                    # quarter of the dump tile; one engine per tile-row.
                    src1 = ps[:, 0 : 2 * N]
                    dst1 = db[:, bank * 2 * N : (bank + 1) * 2 * N]
                    if ltr % 2 == 0:
                        nc.vector.tensor_scalar_mul(dst1, src1, SCALE)
                    else:
                        nc.scalar.mul(dst1, src1, SCALE)
                # store pairs with the drain engine: ACT rows store on
                # scalar (same engine, in-order after its drains); DVE
                # rows store on gpsimd.
                eng = nc.gpsimd if ltr % 2 == 0 else nc.scalar
                eng.dma_start(
                    out=out_d[:, ltr * BPG * N : (ltr + 1) * BPG * N],
                    in_=db,
                )

    nc.compile()
    return nc


def _host_prep(x_b, warped_b):
    """x: [c,h,w] -> tile-contiguous [c, nt_h*nt_w*32]; warped -> padded
    row-major [c, 136*264]."""
    c = x_b.shape[0]
    xt = np.ascontiguousarray(
        x_b.reshape(c, NT_H, TH, NT_W, TW).transpose(0, 1, 3, 2, 4)
    ).reshape(c, NT_H * NT_W * TH * TW)
    wp = np.zeros((c, H + 2 * R, PW), dtype=x_b.dtype)
    wp[:, R : R + H, R : R + W] = warped_b
    return xt, wp.reshape(c, (H + 2 * R) * PW)


def _extract(dump):
    """[128, NGROUPS*BPG*N] -> [81, H, W] constant-stride view.

    dump element [m, g, s*N + n]:
      m = 32*j + hx*TW + wx ; n = (hx+di)*NW + (wx+dj)
      g = ltr ; s = q
      h = ltr*4 + hx ;  w = (q*4+j)*8 + wx
    """
    dmp = np.ascontiguousarray(dump).reshape(128, NGROUPS, BPG * N)
    sm, sg, sn = dmp.strides
    sn_e = sn  # innermost element stride (bytes)
    view = np.lib.stride_tricks.as_strided(
        dmp,
        shape=(K, K, NT_H, TH, 8, TPB, TW),
        #      di dj ltr   hx  q  j   wx
        strides=(
            NW * sn_e,            # di
            sn_e,                 # dj
            sg,                   # ltr
            TW * sm + NW * sn_e,  # hx
            N * sn_e,             # q (= s)
            32 * sm,              # j
            sm + sn_e,            # wx
        ),
    )
    # [di,dj, ltr,hx, q,j,wx] -> [81, H, W]
    out = np.ascontiguousarray(view).reshape(NOFF, H, W)
    return out.astype(np.float32)


def kernel(x, warped):
    from concourse import bass_utils

    x = np.asarray(x, dtype=np.float32)
    warped = np.asarray(warped, dtype=np.float32)
    assert x.shape == (B, C, H, W) and warped.shape == (B, C, H, W)

    import ml_dtypes

    x = x.astype(ml_dtypes.bfloat16)
    warped = warped.astype(ml_dtypes.bfloat16)

    key = "v9"
    if key not in _CACHE:
        _CACHE[key] = _build_module(B)
    nc = _CACHE[key]

    in_maps = []
    for b in range(B):
        xt, wp = _host_prep(x[b], warped[b])
        in_maps.append({"x": xt, "warped": wp})
    res = bass_utils.run_bass_kernel_spmd(nc, in_maps, core_ids=list(range(B)))
    global LAST_RESULTS
    LAST_RESULTS = res
    out = np.empty((B, NOFF, H, W), dtype=np.float32)
    for b in range(B):
        out[b] = _extract(res.results[b]["dump"])
    return out
